# revision 19
# baseline (speedup 1.0000x reference)
"""Trainium2 Bass kernel for nn_BasicBlock (3-layer GCN block with residual).

Math (per batch item b, per conv):
    out = A @ (x @ W) + bias,  A = normalized adjacency (with self loops)
where A[c, r] = sum over edges r->c of dinv[r]*dinv[c] (dense N x N, shared
across batch and precomputed on host from the edge lists).

Block:
    a1 = relu(A_sp @ (x  @ W1) + b1)
    a2 = relu(A_tm @ (a1 @ W2) + b2)
    o3 =      A_sp @ (a2 @ W3) + b3
    out = relu(o3 + x)

Phases per item (matmul forms; AT = A^T so AT[m, n] = A[n, m]):
    1. g1T[c,n]  = sum_m x[m,c]  * AT_sp[m,n]      (lhsT=x chunk,  rhs=AT_sp)
    2. a1T[co,n] = relu(sum_ci W1[ci,co]*g1T[ci,n] + 8*b1)    (8x scaled)
    3. h2[n,c]   = sum_ci a1T[ci,n] * W2[ci,c] / 64
    4. a2T[c,n]  = relu(sum_m h2[m,c]*AT_tm[m,n] + b2)
    5. h3[n,c]   = sum_ci a2T[ci,n] * W3[ci,c] / 8;  h3[N,:] = b3
    6. out[n,c]  = relu(sum_m AT_sp[m,n]*h3[m,c] + x[n,c])
       (AT_sp row N is all-ones over valid cols -> adds b3 to every node;
        harmless in phase 1 because x row N is zero-padded)

All matmuls fp8e4 in DoubleRow perf mode (2 contraction rows per PE cycle,
2x the bf16 rate) with fp32 PSUM accumulation.  Every matmul operand is
stored fp8e4.  The weights are pre-scaled by 8 on the host so their
entries (|w| <= 1/16) stay clear of the e4m3 subnormal range; phase 2
keeps the extra 8x in a1T (bias folded as 8*b1) and the phase-3/5 casts
divide it back out.  End-to-end rel err vs the fp32 reference is ~3e-3.

Scheduling: items are processed in pairs with the two items' phases
interleaved (A.p1 B.p1 A.p2 B.p2 ...) so each phase-boundary stall (PE
waiting on the sibling engine's PSUM->SBUF cast) is covered by the other
item's independent matmuls.  PSUM->SBUF work alternates between the
vector / scalar / gpsimd engines per item parity.  Phase 6 shares the
AT_sp stationary operand across the pair (F=512 matmuls), adds the bf16
residual from a pair-interleaved copy of x, and stores two big pair-tiled
chunks per pair on the sync ring.  Batch (64) is sharded 8 items/core.
"""

import sys

if "/opt/trn_rl_repo" not in sys.path:
    sys.path.insert(0, "/opt/trn_rl_repo")

import numpy as np
import ml_dtypes

import concourse.bass as bass
import concourse.bacc as bacc
import concourse.mybir as mybir
import concourse.tile as tile
from concourse.bass_utils import run_bass_kernel_spmd

P = 128
B, N, C = 64, 1700, 256
N_CORES = 8
B_LOCAL = B // N_CORES
W_SCALE = 8.0

F32 = mybir.dt.float32
BF16 = mybir.dt.bfloat16
E4 = mybir.dt.float8e4
RELU = mybir.ActivationFunctionType.Relu
DR = mybir.MatmulPerfMode.DoubleRow
ADD = mybir.AluOpType.add
MAX = mybir.AluOpType.max
NP_BF16 = ml_dtypes.bfloat16
NP_E4 = ml_dtypes.float8_e4m3


def _quarters(total, step=512):
    return [(q, min(step, total - q)) for q in range(0, total, step)]


def build_program(bl, n, c):
    """Build the Bass/Tile program for `bl` batch items, `n` nodes, `c` chans."""
    kt = -(-(n + 1) // P)  # node chunks; >= one pad row (bias row at index n)
    assert kt % 2 == 0, "DoubleRow pairing needs an even node-chunk count"
    assert bl % 2 == 0
    npad = kt * P
    kt2 = kt // 2
    ct = c // P
    bl2 = bl // 2

    nqv = _quarters(n)  # valid-column quarters (phases whose pads are unread)

    nc = bacc.Bacc("TRN2", target_bir_lowering=False, debug=False,
                   enable_asserts=False)

    x8_d = nc.dram_tensor("x8pair", [P, bl2, kt * 2 * c], E4,
                          kind="ExternalInput")
    xpr_d = nc.dram_tensor("xpair", [P, bl2, kt * 2 * c], BF16,
                           kind="ExternalInput")
    atsp_d = nc.dram_tensor("at_sp", [P, kt, n], E4, kind="ExternalInput")
    attm_d = nc.dram_tensor("at_tm", [P, kt, n], E4, kind="ExternalInput")
    w_d = [nc.dram_tensor(f"w{i}", [P, ct, c], E4, kind="ExternalInput")
           for i in (1, 2, 3)]
    b1_d = nc.dram_tensor("b1", [P, ct], F32, kind="ExternalInput")  # 8*b1
    b2_d = nc.dram_tensor("b2", [P, ct], F32, kind="ExternalInput")
    b3_d = nc.dram_tensor("b3", [1, c], E4, kind="ExternalInput")
    out_d = nc.dram_tensor("out", [P, bl2, kt * 2 * c], BF16,
                           kind="ExternalOutput")

    with tile.TileContext(nc) as tc:
        with (
            tc.tile_pool(name="const", bufs=1) as cpool,
            tc.tile_pool(name="xpr", bufs=2) as xprp,
            tc.tile_pool(name="x8", bufs=2) as x8p,
            tc.tile_pool(name="act", bufs=4) as actp,
            tc.tile_pool(name="h", bufs=2) as hp,
            tc.tile_pool(name="hpair", bufs=2) as hpp,
            tc.tile_pool(name="outp", bufs=2) as outp,
            tc.tile_pool(name="psA", bufs=4, space="PSUM") as psA,
            tc.tile_pool(name="psW", bufs=4, space="PSUM") as psW,
        ):
            # --- constants.  at_sp is needed first (item-0 phase 1
            # consumes tile pair k2 early), so every tile is split across
            # the sync+scalar HWDGE rings, with at_tm queued behind it.
            # Row pitch npad (not n): DoubleRow LDWEIGHTS/matmul require
            # the slot stride to be a 16B multiple (1700 % 16 != 0, 1792
            # ok); cols [n:npad] are never read by any matmul. ---
            at_sp = cpool.tile([P, kt, npad], E4, tag="at_sp")
            at_tm = cpool.tile([P, kt, npad], E4, tag="at_tm")
            nh = n // 2
            for k2 in range(kt2):
                ks = slice(2 * k2, 2 * k2 + 2)
                if k2 == 0:
                    # finer column chunks so item-0's first chain can
                    # start ~0.6us in
                    for (q0, qs) in _quarters(n, 448):
                        eng = nc.sync if (q0 // 448) % 2 == 0 else nc.scalar
                        eng.dma_start(at_sp[:, ks, q0:q0 + qs],
                                      atsp_d[:, ks, q0:q0 + qs])
                else:
                    nc.sync.dma_start(at_sp[:, ks, :nh], atsp_d[:, ks, :nh])
                    nc.scalar.dma_start(at_sp[:, ks, nh:n], atsp_d[:, ks, nh:])

            w_sb = []
            for i, wd in enumerate(w_d):
                w = cpool.tile([P, ct, c], E4, tag=f"w{i}")
                nc.sync.dma_start(w[:], wd[:])
                w_sb.append(w)
            b1_sb = cpool.tile([P, ct], F32, tag="b1")
            b2_sb = cpool.tile([P, ct], F32, tag="b2")
            nc.sync.dma_start(b1_sb[:], b1_d[:])
            nc.sync.dma_start(b2_sb[:], b2_d[:])

            def emit_load_at_tm():
                # queued on the rings behind at_sp -- needed only from
                # item-0 phase 4
                for k2 in range(kt2):
                    ks = slice(2 * k2, 2 * k2 + 2)
                    nc.sync.dma_start(at_tm[:, ks, :nh], attm_d[:, ks, :nh])
                    nc.scalar.dma_start(at_tm[:, ks, nh:n], attm_d[:, ks, nh:])

            bias_tile = n // P      # global node index n == first pad row
            bias_part = n % P

            def emit_load_x8(j):
                # host delivers x pre-tiled pair-interleaved [P, kt*2c]
                # fp8 (pad rows zeroed); phase 1 slices one item's columns
                # out of it.  Pair 0 feeds the streamed phase-1 variant:
                # load in k2-pair chunks so the first matmul starts early.
                x8 = x8p.tile([P, kt, 2 * c], E4, tag="x8", name=f"x8_{j}")
                if j == 0:
                    for k2 in range(kt2):
                        nc.gpsimd.dma_start(
                            x8[:, 2 * k2:2 * k2 + 2, :],
                            x8_d[:, j, k2 * 4 * c:(k2 + 1) * 4 * c])
                else:
                    nc.gpsimd.dma_start(x8[:, :, :], x8_d[:, j, :])
                return x8

            COPY = mybir.ActivationFunctionType.Copy

            def emit_p1(b, x8):
                ipc = (b % 2) * c
                # phase 1: g1T = (A_sp @ x)^T
                g1T = actp.tile([P, ct, npad], E4, tag="act", name=f"g1T_{b}")
                cnt = [b]

                def cast(dst, src):
                    # alternate finisher queues so PSUM rotation never
                    # convoys behind a single backed-up engine
                    cnt[0] += 1
                    if cnt[0] % 2 == 0:
                        nc.vector.tensor_copy(dst, src)
                    else:
                        nc.scalar.activation(dst, src, COPY)
                if b <= 1:
                    # k2-outer over 8 parallel PSUM banks so at_sp tile
                    # pairs are consumed as soon as their DMAs land
                    groups = []
                    for cc in range(ct):
                        for qi, (q0, qs) in enumerate(nqv):
                            pool, tg = ((psA, "psA")
                                        if (cc * len(nqv) + qi) % 2 == 0
                                        else (psW, "psW"))
                            groups.append(
                                (pool.tile([P, 512], F32, tag=tg,
                                           name=f"ps1_{cc}_{qi}"), cc, q0, qs))
                    for k2 in range(kt2):
                        for (ps, cc, q0, qs) in groups:
                            nc.tensor.matmul(
                                ps[:, :qs],
                                lhsT=x8[:, 2 * k2:2 * k2 + 2,
                                        ipc + cc * P:ipc + (cc + 1) * P],
                                rhs=at_sp[:, 2 * k2:2 * k2 + 2, q0:q0 + qs],
                                start=(k2 == 0), stop=(k2 == kt2 - 1),
                                perf_mode=DR)
                    for (ps, cc, q0, qs) in groups:
                        cast(g1T[:, cc, q0:q0 + qs], ps[:, :qs])
                else:
                    for cc in range(ct):
                        for (q0, qs) in nqv:
                            ps = psA.tile([P, 512], F32, tag="psA")
                            for k2 in range(kt2):
                                nc.tensor.matmul(
                                    ps[:, :qs],
                                    lhsT=x8[:, 2 * k2:2 * k2 + 2,
                                            ipc + cc * P:ipc + (cc + 1) * P],
                                    rhs=at_sp[:, 2 * k2:2 * k2 + 2, q0:q0 + qs],
                                    start=(k2 == 0), stop=(k2 == kt2 - 1),
                                    perf_mode=DR)
                            cast(g1T[:, cc, q0:q0 + qs], ps[:, :qs])
                return g1T

            def emit_p2(b, g1T):
                # phase 2: a1T = relu(W1^T @ g1T + 8*b1)  (8x-scaled a1)
                a1T = actp.tile([P, ct, npad], E4, tag="act", name=f"a1T_{b}")
                for cc in range(ct):
                    # cols [n:npad] are read as phase-3 lhsT pads but never
                    # written by the trimmed quarters
                    nc.vector.memset(a1T[:, cc, n:npad], 0)
                idx = b
                for co in range(ct):
                    for (q0, qs) in nqv:
                        ps = psA.tile([P, 512], F32, tag="psA")
                        nc.tensor.matmul(
                            ps[:, :qs],
                            lhsT=w_sb[0][:, 0:2, co * P:(co + 1) * P],
                            rhs=g1T[:, 0:2, q0:q0 + qs],
                            start=True, stop=True, perf_mode=DR)
                        idx += 1
                        if idx % 2 == 0:
                            nc.vector.tensor_scalar(
                                a1T[:, co, q0:q0 + qs], ps[:, :qs],
                                b1_sb[:, co:co + 1], 0.0, ADD, MAX)
                        else:
                            nc.scalar.activation(
                                a1T[:, co, q0:q0 + qs], ps[:, :qs], RELU,
                                bias=b1_sb[:, co:co + 1])
                return a1T

            def emit_p3(b, a1T):
                # phase 3: h2 = a1 @ W2 (natural layout); psum holds
                # 64*(a1@W2): a1T is 8x and W2 is 8x
                h2 = hp.tile([P, kt, c], E4, tag="h", name=f"h2_{b}")
                for k2 in range(kt2):
                    ps = psW.tile([P, 2 * c], F32, tag="psW")
                    for j in range(2):
                        k = 2 * k2 + j
                        nc.tensor.matmul(
                            ps[:, j * c:(j + 1) * c],
                            lhsT=a1T[:, 0:2, k * P:(k + 1) * P],
                            rhs=w_sb[1][:, 0:2, :],
                            start=True, stop=True, perf_mode=DR)
                    dst = h2[:, 2 * k2:2 * k2 + 2, :]
                    if (b + k2) % 2 == 0:
                        nc.vector.tensor_scalar_mul(dst, ps[:], 1.0 / 64.0)
                    else:
                        nc.scalar.activation(dst, ps[:], COPY,
                                             scale=1.0 / 64.0)
                return h2

            def emit_p4(b, h2):
                # phase 4: a2T = relu((A_tm @ h2)^T + b2)
                a2T = actp.tile([P, ct, npad], E4, tag="act", name=f"a2T_{b}")
                for cc in range(ct):
                    nc.vector.memset(a2T[:, cc, n:npad], 0)

                fcnt = [b]

                def finish(dst, ps):
                    fcnt[0] += 1
                    if fcnt[0] % 2 == 0:
                        nc.scalar.activation(dst, ps, RELU,
                                             bias=b2_sb[:, cc:cc + 1])
                    else:
                        nc.vector.tensor_scalar(dst, ps,
                                                b2_sb[:, cc:cc + 1], 0.0,
                                                ADD, MAX)

                if b == 0:
                    # k2-outer in two 4-bank rounds so at_tm tile pairs are
                    # consumed while their DMAs are still landing
                    for cc in range(ct):
                        groups = [(psA.tile([P, 512], F32, tag="psA",
                                            name=f"ps4_{cc}_{q0}"), q0, qs)
                                  for (q0, qs) in nqv]
                        for k2 in range(kt2):
                            for (ps, q0, qs) in groups:
                                nc.tensor.matmul(
                                    ps[:, :qs],
                                    lhsT=h2[:, 2 * k2:2 * k2 + 2,
                                            cc * P:(cc + 1) * P],
                                    rhs=at_tm[:, 2 * k2:2 * k2 + 2, q0:q0 + qs],
                                    start=(k2 == 0), stop=(k2 == kt2 - 1),
                                    perf_mode=DR)
                        for (ps, q0, qs) in groups:
                            finish(a2T[:, cc, q0:q0 + qs], ps[:, :qs])
                else:
                    for cc in range(ct):
                        for (q0, qs) in nqv:
                            ps = psA.tile([P, 512], F32, tag="psA")
                            for k2 in range(kt2):
                                nc.tensor.matmul(
                                    ps[:, :qs],
                                    lhsT=h2[:, 2 * k2:2 * k2 + 2,
                                            cc * P:(cc + 1) * P],
                                    rhs=at_tm[:, 2 * k2:2 * k2 + 2, q0:q0 + qs],
                                    start=(k2 == 0), stop=(k2 == kt2 - 1),
                                    perf_mode=DR)
                            finish(a2T[:, cc, q0:q0 + qs], ps[:, :qs])
                return a2T

            def emit_p5_pair(b, a2T, h3p, ip):
                # phase 5: h3 = a2 @ W3 into flat slot ip of a 2-item h3;
                # psum holds 8*(a2@W3) (W3 is 8x); h3[row n] = b3
                for k2 in range(kt2):
                    ps = psW.tile([P, 2 * c], F32, tag="psW")
                    for j in range(2):
                        k = 2 * k2 + j
                        nc.tensor.matmul(
                            ps[:, j * c:(j + 1) * c],
                            lhsT=a2T[:, 0:2, k * P:(k + 1) * P],
                            rhs=w_sb[2][:, 0:2, :],
                            start=True, stop=True, perf_mode=DR)
                    dst = h3p[:, 2 * k2:2 * k2 + 2, ip * c:(ip + 1) * c]
                    if (b + k2) % 2 == 0:
                        nc.scalar.activation(dst, ps[:], COPY,
                                             scale=1.0 / W_SCALE)
                    else:
                        nc.vector.tensor_scalar_mul(dst, ps[:],
                                                    1.0 / W_SCALE)
                nc.sync.dma_start(
                    h3p[bias_part:bias_part + 1, bias_tile,
                        ip * c:(ip + 1) * c], b3_d[:, :])

            def emit_p6_pair(j, xpr, h3p):
                # phase 6 over the pair: F=512 single-pass matmuls sharing
                # the AT_sp stationary operand; residual added from the
                # pair-interleaved bf16 x; relu once per kt/2 chunk; two
                # pair-tiled stores on the sync ring
                opair = outp.tile([P, kt, 2 * c], BF16, tag="o",
                                  name=f"opair_{j}")
                bounds = [2, 4, 6, 8, 10, 12, 13, kt]  # store chunk ends
                h0 = 0
                for ko in range(kt):
                    rows = min(P, n - ko * P)
                    if rows > 0:
                        ps = psW.tile([P, 2 * c], F32, tag="psW")
                        for k2 in range(kt2):
                            nc.tensor.matmul(
                                ps[:rows, :],
                                lhsT=at_sp[:, 2 * k2:2 * k2 + 2,
                                           ko * P:ko * P + rows],
                                rhs=h3p[:, 2 * k2:2 * k2 + 2, :],
                                start=(k2 == 0), stop=(k2 == kt2 - 1),
                                perf_mode=DR)
                        nc.vector.tensor_add(opair[:rows, ko, :],
                                             ps[:rows, :], xpr[:rows, ko, :])
                    if ko + 1 in bounds:
                        sl = opair[:, h0:ko + 1, :]
                        nc.scalar.activation(sl, sl, RELU)
                        nc.sync.dma_start(
                            out_d[:, j, h0 * 2 * c:(ko + 1) * 2 * c], sl)
                        h0 = ko + 1

            def emit_pair_mid(j, g1Ta, g1Tb):
                # residual loads ride gpsimd here: needed only at phase 6,
                # keeping the prologue rings clear for at_sp/at_tm
                xpr = xprp.tile([P, kt, 2 * c], BF16, tag="xpr",
                                name=f"xpr_{j}")
                nc.gpsimd.dma_start(xpr[:, :, :], xpr_d[:, j, :])
                # interleaved schedule: A.p2 B.p2 A.p3 B.p3 ... so every
                # phase-boundary cast latency is covered by the sibling
                # item's independent matmuls
                ba, bb = 2 * j, 2 * j + 1
                a1Ta = emit_p2(ba, g1Ta)
                a1Tb = emit_p2(bb, g1Tb)
                h2a = emit_p3(ba, a1Ta)
                h2b = emit_p3(bb, a1Tb)
                a2Ta = emit_p4(ba, h2a)
                a2Tb = emit_p4(bb, h2b)
                h3p = hpp.tile([P, kt, 2 * c], E4, tag="hpair",
                               name=f"h3p_{j}")
                emit_p5_pair(ba, a2Ta, h3p, 0)
                emit_p5_pair(bb, a2Tb, h3p, 1)
                return xpr, h3p

            # Pairs are software-pipelined by one stage: pair j's phase 6
            # is emitted after pair j+1's two phase-1 blocks, so the
            # p5->p6 and p1->p2 cast latencies are both covered by
            # independent matmul streams.
            x8 = emit_load_x8(0)
            g1Ta = emit_p1(0, x8)
            emit_load_at_tm()
            g1Tb = emit_p1(1, x8)
            pending = None  # (j, xpr, h3p) awaiting phase 6
            for j in range(bl2):
                xpr, h3p = emit_pair_mid(j, g1Ta, g1Tb)
                pending = (j, xpr, h3p)
                if j + 1 < bl2:
                    x8 = emit_load_x8(j + 1)
                    g1Ta = emit_p1(2 * (j + 1), x8)
                    g1Tb = emit_p1(2 * (j + 1) + 1, x8)
                emit_p6_pair(*pending)

    nc.compile()
    return nc


def _norm_adj_T(edges, n, npad, bias_row):
    """A^T padded to [npad, npad] in fp32. AT[m, j] = A[j, m] where
    out[j] += A[j, m] * h[m]; edge (r -> c) contributes dinv[r]*dinv[c] at
    AT[r, c]. Self loops included. If bias_row, AT[n, :n] = 1 (bias fold)."""
    row = np.concatenate([edges[0], np.arange(n, dtype=np.int64)])
    col = np.concatenate([edges[1], np.arange(n, dtype=np.int64)])
    deg = np.bincount(col, minlength=n).astype(np.float32)
    dinv = np.zeros(n, np.float32)
    nz = deg > 0
    dinv[nz] = 1.0 / np.sqrt(deg[nz])
    norm = dinv[row] * dinv[col]
    at = np.zeros((npad, npad), np.float32)
    np.add.at(at, (row, col), norm)
    if bias_row:
        at[n, :n] = 1.0
    return at


def _tile_rows(a, kt):
    """[kt*P, F] -> [P, kt, F] so that [p, k, :] = a[k*P + p, :]."""
    return np.ascontiguousarray(
        a.reshape(kt, P, a.shape[-1]).transpose(1, 0, 2))


_PROGRAM_CACHE = {}


def _get_program(bl, n, c):
    key = (bl, n, c)
    if key not in _PROGRAM_CACHE:
        _PROGRAM_CACHE[key] = build_program(bl, n, c)
    return _PROGRAM_CACHE[key]


def run(inputs, trace=False, n_cores=N_CORES):
    x = np.asarray(inputs["x"], dtype=np.float32)
    w1 = np.asarray(inputs["W1"], np.float32)
    w2 = np.asarray(inputs["W2"], np.float32)
    w3 = np.asarray(inputs["W3"], np.float32)
    b1 = np.asarray(inputs["b1"], np.float32)
    b2 = np.asarray(inputs["b2"], np.float32)
    b3 = np.asarray(inputs["b3"], np.float32)
    e_sp = np.asarray(inputs["keypoint_line_without_temporal"]).astype(np.int64)
    e_tm = np.asarray(inputs["keypoint_line_with_temporal"]).astype(np.int64)

    b_total, n, c = x.shape
    bl = b_total // n_cores
    bl2 = bl // 2
    kt = -(-(n + 1) // P)
    npad = kt * P
    ct = c // P

    nc = _get_program(bl, n, c)

    at_sp = _tile_rows(
        _norm_adj_T(e_sp, n, npad, bias_row=True)[:, :n].astype(NP_E4), kt)
    at_tm = _tile_rows(
        _norm_adj_T(e_tm, n, npad, bias_row=False)[:, :n].astype(NP_E4), kt)

    # x pre-tiled: [b, n, c] -> pad to [b, npad, c] -> [P, b, kt, c]
    xpad = np.zeros((b_total, npad, c), np.float32)
    xpad[:, :n, :] = x
    xt = xpad.reshape(b_total, kt, P, c).transpose(2, 0, 1, 3)  # [P,b,kt,c]
    # pair-interleaved [P, npairs, kt, 2, c]: fp8 feeds phases 1/6,
    # bf16 is the phase-6 residual
    xt5 = np.ascontiguousarray(
        xt.reshape(P, b_total // 2, 2, kt, c).transpose(0, 1, 3, 2, 4)
        .reshape(P, b_total // 2, kt * 2 * c))
    x8pr = xt5.astype(NP_E4)
    xpr = xt5.astype(NP_BF16)

    shared = {
        "at_sp": at_sp,
        "at_tm": at_tm,
        "w1": _tile_rows((w1 * W_SCALE).astype(NP_E4), ct),
        "w2": _tile_rows((w2 * W_SCALE).astype(NP_E4), ct),
        "w3": _tile_rows((w3 * W_SCALE).astype(NP_E4), ct),
        "b1": np.ascontiguousarray((b1 * W_SCALE).reshape(ct, P).T),
        "b2": np.ascontiguousarray(b2.reshape(ct, P).T),
        "b3": np.ascontiguousarray(b3.astype(NP_E4)[None, :]),
    }
    in_maps = [
        {"x8pair": np.ascontiguousarray(x8pr[:, i * bl2:(i + 1) * bl2]),
         "xpair": np.ascontiguousarray(xpr[:, i * bl2:(i + 1) * bl2]),
         **shared}
        for i in range(n_cores)
    ]
    res = run_bass_kernel_spmd(nc, in_maps, core_ids=list(range(n_cores)),
                               trace=trace)
    outs = []
    for r in res.results:
        o = np.asarray(r["out"], np.float32)  # [P, bl2, kt*2c]
        o = o.reshape(P, bl2, kt, 2, c).transpose(1, 3, 2, 0, 4)
        o = o.reshape(bl, npad, c)[:, :n, :]
        outs.append(o)
    out = np.concatenate(outs, axis=0)
    return out, res


def kernel(**inputs) -> np.ndarray:
    out, _ = run(inputs, trace=False)
    return out
